# revision 1
# baseline (speedup 1.0000x reference)
"""GCN layer (DGL GraphConv norm='both' + relu + residual + LayerNorm) on 8 trn2 NeuronCores.

Original baseline (per-tile indirect gathers). Kept for device health checks.
"""

import numpy as np


def _ensure_path():
    try:
        import concourse  # noqa: F401
    except ImportError:
        import sys

        for p in ("/opt/trn_rl_repo", "/root/.axon_site/_ro/trn_rl_repo"):
            if p not in sys.path:
                sys.path.insert(0, p)


P = 128
LN_EPS = 1e-5


# ---------------------------------------------------------------- host prep
def host_prep(feats, src, dst, W, b, gamma, beta, n_cores):
    N, D = feats.shape
    assert N % n_cores == 0
    npc = N // n_cores                      # nodes per core
    nblk = (npc + P - 1) // P               # 128-node blocks per core
    rows_pp = (N + 1 + P - 1) // P          # table rows per partition
    npad = rows_pp * P                      # padded table rows (>= N+1)
    zero_row = N                            # an all-zero table row for padding

    src = np.asarray(src).astype(np.int64)
    dst = np.asarray(dst).astype(np.int64)

    feats_pad = np.zeros((npad, D), np.float32)
    feats_pad[:N] = feats

    order = np.argsort(dst, kind="stable")
    src_s = src[order]
    dst_s = dst[order]
    rp_dst = np.searchsorted(dst_s, np.arange(N + 1)).astype(np.int64)
    rp_src = np.searchsorted(np.sort(src), np.arange(npad + 1)).astype(np.int32)

    # per (core, block) edge counts -> shared tile schedule K[j]
    cnts = np.zeros((n_cores, nblk), np.int64)
    for m in range(n_cores):
        base = m * npc
        loc = rp_dst[base : base + npc + 1]
        lo = loc[np.minimum(np.arange(nblk) * P, npc)]
        hi = loc[np.minimum((np.arange(nblk) + 1) * P, npc)]
        cnts[m] = hi - lo
    K = np.maximum(1, (-(-cnts // P)).max(axis=0)).astype(np.int64)  # [nblk]
    C = np.concatenate([[0], np.cumsum(K)]).astype(np.int64)
    tot_k = int(C[-1])

    gidx = np.full((n_cores, P, tot_k), zero_row, np.int32)
    dstcol = np.full((n_cores, P, tot_k), -1.0, np.float32)
    for m in range(n_cores):
        base = m * npc
        for j in range(nblk):
            e0 = rp_dst[base + min(j * P, npc)]
            cnt = int(cnts[m, j])
            kj = int(K[j])
            bi = np.full(kj * P, zero_row, np.int64)
            bd = np.full(kj * P, -1.0, np.float32)
            bi[:cnt] = src_s[e0 : e0 + cnt]
            bd[:cnt] = dst_s[e0 : e0 + cnt] - (base + j * P)
            gidx[m, :, C[j] : C[j + 1]] = bi.reshape(kj, P).T
            dstcol[m, :, C[j] : C[j + 1]] = bd.reshape(kj, P).T

    # per-core local dst rowptr, padded to nblk*P+1 entries
    rp_dst_mine = np.zeros((n_cores, nblk * P + 1), np.int32)
    for m in range(n_cores):
        base = m * npc
        loc = rp_dst[base : base + npc + 1]
        rp_dst_mine[m, : npc + 1] = loc
        rp_dst_mine[m, npc + 1 :] = loc[-1]

    iota = np.tile(np.arange(P, dtype=np.float16), (P, 1))
    ident = np.eye(D, dtype=np.float32)

    in_maps = []
    for m in range(n_cores):
        base = m * npc
        in_maps.append(
            {
                "feats_pad": feats_pad,
                "feats_mine": np.ascontiguousarray(feats[base : base + npc]).astype(np.float32),
                "rp_src": rp_src,
                "rp_dst_mine": np.ascontiguousarray(rp_dst_mine[m]),
                "gidx": np.ascontiguousarray(gidx[m]),
                "dstcol": np.ascontiguousarray(dstcol[m]),
                "Wmat": np.asarray(W, np.float32),
                "bvec": np.asarray(b, np.float32),
                "gamma": np.asarray(gamma, np.float32),
                "beta": np.asarray(beta, np.float32),
                "iota": iota,
                "ident": ident,
            }
        )

    meta = dict(
        N=N, D=D, n_cores=n_cores, npc=npc, nblk=nblk, rows_pp=rows_pp,
        npad=npad, K=[int(k) for k in K], C=[int(c) for c in C], tot_k=tot_k,
    )
    return in_maps, meta


def _split_multiwaits(nc, mybir):
    """This walrus build allows only one sync-wait per instruction; hoist
    extra waits onto same-engine NoOps placed just before the instruction."""
    n = 0
    for f in nc.m.functions:
        for bb in f.blocks:
            newlist = []
            for inst in bb.instructions:
                si = getattr(inst, "sync_info", None)
                if si is not None and len(si.on_wait) > 1:
                    waits = list(si.on_wait)
                    for w in waits[:-1]:
                        nop = mybir.InstNoOp(name=f"I-WS-{n}", ins=[], outs=[])
                        n += 1
                        nop.engine = inst.engine
                        nop.sync_info = mybir.SyncInfo(on_wait=[w], on_update=[])
                        newlist.append(nop)
                    inst.sync_info = mybir.SyncInfo(
                        on_wait=[waits[-1]], on_update=list(si.on_update)
                    )
                newlist.append(inst)
            bb.instructions = newlist


# ---------------------------------------------------------------- device program
def build_nc(meta, debug=False, split_waits=True):
    _ensure_path()
    from contextlib import ExitStack

    import concourse.bass as bass
    import concourse.tile as tile
    from concourse import mybir

    dt = mybir.dt
    f32, f16, i32 = dt.float32, dt.float16, dt.int32
    Alu = mybir.AluOpType
    Act = mybir.ActivationFunctionType

    N = meta["N"]
    D = meta["D"]
    npc = meta["npc"]
    nblk = meta["nblk"]
    rows_pp = meta["rows_pp"]
    npad = meta["npad"]
    K = meta["K"]
    C = meta["C"]
    tot_k = meta["tot_k"]

    nc = bass.Bass()

    feats_pad = nc.declare_dram_parameter("feats_pad", [npad, D], f32, isOutput=False)
    feats_mine = nc.declare_dram_parameter("feats_mine", [npc, D], f32, isOutput=False)
    rp_src = nc.declare_dram_parameter("rp_src", [npad + 1], i32, isOutput=False)
    rp_dst_mine = nc.declare_dram_parameter("rp_dst_mine", [nblk * P + 1], i32, isOutput=False)
    gidx_in = nc.declare_dram_parameter("gidx", [P, tot_k], i32, isOutput=False)
    dstcol_in = nc.declare_dram_parameter("dstcol", [P, tot_k], f32, isOutput=False)
    W_in = nc.declare_dram_parameter("Wmat", [D, D], f32, isOutput=False)
    b_in = nc.declare_dram_parameter("bvec", [D], f32, isOutput=False)
    gamma_in = nc.declare_dram_parameter("gamma", [D], f32, isOutput=False)
    beta_in = nc.declare_dram_parameter("beta", [D], f32, isOutput=False)
    iota_in = nc.declare_dram_parameter("iota", [P, P], f16, isOutput=False)
    ident_in = nc.declare_dram_parameter("ident", [D, D], f32, isOutput=False)
    out_t = nc.declare_dram_parameter("out", [npc, D], f32, isOutput=True)

    h_dram = nc.dram_tensor("h_table", [npad, D], f16)
    din_scr = nc.dram_tensor("din_scr", [nblk * P], f32)

    def bcast_row(ap, parts):
        # [L] dram vector -> [parts, L] partition-broadcast access pattern
        return bass.AP(tensor=ap.tensor, offset=ap.offset, ap=[[0, parts]] + list(ap.ap))

    with ExitStack() as ctx:
        tc = ctx.enter_context(tile.TileContext(nc))
        const = ctx.enter_context(tc.tile_pool(name="const", bufs=1))
        ph1 = ctx.enter_context(tc.tile_pool(name="ph1", bufs=2))
        gp = ctx.enter_context(tc.tile_pool(name="gp", bufs=12))
        indp = ctx.enter_context(tc.tile_pool(name="indp", bufs=8))
        ep = ctx.enter_context(tc.tile_pool(name="ep", bufs=3))
        pp = ctx.enter_context(tc.tile_pool(name="pp", bufs=2, space="PSUM"))

        # ---- constants -------------------------------------------------
        iota_sb = const.tile([P, P], f16)
        nc.sync.dma_start(out=iota_sb[:], in_=iota_in[:])
        ident_sb = const.tile([D, D], f32)
        nc.sync.dma_start(out=ident_sb[:], in_=ident_in[:])
        w_f32 = const.tile([D, D], f32)
        nc.sync.dma_start(out=w_f32[:], in_=W_in[:])
        w_sb = const.tile([D, D], f16)
        nc.vector.tensor_copy(out=w_sb[:], in_=w_f32[:])
        b_bc = const.tile([P, D], f32)
        nc.sync.dma_start(out=b_bc[:], in_=bcast_row(b_in[:], P))
        gamma_bc = const.tile([P, D], f32)
        nc.sync.dma_start(out=gamma_bc[:], in_=bcast_row(gamma_in[:], P))
        beta_bc = const.tile([P, D], f32)
        nc.sync.dma_start(out=beta_bc[:], in_=bcast_row(beta_in[:], P))
        eps_sb = const.tile([P, 1], f32)
        nc.vector.memset(eps_sb[:], LN_EPS)
        gidx_sb = const.tile([P, tot_k], i32)
        nc.sync.dma_start(out=gidx_sb[:], in_=gidx_in[:])
        dstcol_sb = const.tile([P, tot_k], f32)
        nc.sync.dma_start(out=dstcol_sb[:], in_=dstcol_in[:])

        # ---- rsqrt(deg_out) (global, p-major) --------------------------
        rpo = ph1.tile([P, rows_pp + 1], i32, tag="rp")
        nc.sync.dma_start(
            out=rpo[:],
            in_=bass.AP(tensor=rp_src[:].tensor, offset=0, ap=[[rows_pp, P], [1, rows_pp + 1]]),
        )
        deg_i = ph1.tile([P, rows_pp], i32, tag="degi")
        nc.vector.tensor_tensor(
            out=deg_i[:], in0=rpo[:, 1 : rows_pp + 1], in1=rpo[:, 0:rows_pp], op=Alu.subtract
        )
        dgo = const.tile([P, rows_pp], f32)  # persists through phase 1
        nc.vector.tensor_copy(out=dgo[:], in_=deg_i[:])
        nc.vector.tensor_scalar_max(out=dgo[:], in0=dgo[:], scalar1=1.0)
        nc.scalar.sqrt(out=dgo[:], in_=dgo[:])
        nc.vector.reciprocal(out=dgo[:], in_=dgo[:])

        # ---- rsqrt(deg_in) for my nodes -> din_scr (node order) --------
        rpdo = ph1.tile([P, nblk + 1], i32, tag="rpd")
        nc.sync.dma_start(
            out=rpdo[:],
            in_=bass.AP(tensor=rp_dst_mine[:].tensor, offset=0, ap=[[nblk, P], [1, nblk + 1]]),
        )
        din_i = ph1.tile([P, nblk], i32, tag="dini")
        nc.vector.tensor_tensor(
            out=din_i[:], in0=rpdo[:, 1 : nblk + 1], in1=rpdo[:, 0:nblk], op=Alu.subtract
        )
        din_f = ph1.tile([P, nblk], f32, tag="dinf")
        nc.vector.tensor_copy(out=din_f[:], in_=din_i[:])
        nc.vector.tensor_scalar_max(out=din_f[:], in0=din_f[:], scalar1=1.0)
        nc.scalar.sqrt(out=din_f[:], in_=din_f[:])
        nc.vector.reciprocal(out=din_f[:], in_=din_f[:])
        nc.sync.dma_start(out=din_scr[:].rearrange("(p r) -> p r", p=P), in_=din_f[:])

        # ---- phase 1: h table = fp16(feats * rsqrt(deg_out)) -----------
        fview = feats_pad[:].rearrange("(p r) d -> p r d", p=P)
        hview = h_dram[:].rearrange("(p r) d -> p r d", p=P)
        n_chunks = 8
        cw = -(-rows_pp // n_chunks)
        for c in range(n_chunks):
            r0 = c * cw
            r1 = min(r0 + cw, rows_pp)
            if r0 >= r1:
                break
            w_ = r1 - r0
            ft = ph1.tile([P, cw, D], f32, tag="ft")
            nc.sync.dma_start(out=ft[:, :w_, :], in_=fview[:, r0:r1, :])
            ht = ph1.tile([P, cw, D], f16, tag="ht")
            for r in range(w_):
                nc.vector.tensor_scalar(
                    out=ht[:, r, :],
                    in0=ft[:, r, :],
                    scalar1=dgo[:, r0 + r : r0 + r + 1],
                    scalar2=None,
                    op0=Alu.mult,
                )
            nc.sync.dma_start(out=hview[:, r0:r1, :], in_=ht[:, :w_, :])

        tc.strict_bb_all_engine_barrier()

        # ---- phase 2: per-block aggregation + epilogue -----------------
        for j in range(nblk):
            kj = K[j]
            bs = min(P, npc - j * P)
            agg_ps = pp.tile([D, P], f32, tag="agg")
            for k in range(kj):
                g = gp.tile([P, D], f16, tag="g")
                nc.gpsimd.indirect_dma_start(
                    out=g[:],
                    out_offset=None,
                    in_=h_dram[:],
                    in_offset=bass.IndirectOffsetOnAxis(
                        ap=gidx_sb[:, C[j] + k : C[j] + k + 1], axis=0
                    ),
                )
                ind = indp.tile([P, P], f16, tag="ind")
                nc.vector.tensor_scalar(
                    out=ind[:],
                    in0=iota_sb[:],
                    scalar1=dstcol_sb[:, C[j] + k : C[j] + k + 1],
                    scalar2=None,
                    op0=Alu.is_equal,
                )
                nc.tensor.matmul(
                    out=agg_ps[:],
                    lhsT=g[:],
                    rhs=ind[:],
                    start=(k == 0),
                    stop=(k == kj - 1),
                )
            agg_sb = ep.tile([D, P], f16, tag="aggsb")
            nc.vector.tensor_copy(out=agg_sb[:], in_=agg_ps[:])
            w_ps = pp.tile([D, P], f32, tag="wps")
            nc.tensor.matmul(out=w_ps[:], lhsT=w_sb[:], rhs=agg_sb[:], start=True, stop=True)
            w_sbuf = ep.tile([D, P], f32, tag="wsb")
            nc.vector.tensor_copy(out=w_sbuf[:], in_=w_ps[:])
            t_ps = pp.tile([P, D], f32, tag="tps")
            nc.tensor.transpose(out=t_ps[:], in_=w_sbuf[:], identity=ident_sb[:])

            din = ep.tile([P, 1], f32, tag="din")
            nc.sync.dma_start(out=din[:], in_=din_scr[j * P : (j + 1) * P, None])
            x = ep.tile([P, D], f32, tag="x")
            nc.vector.tensor_scalar(
                out=x[:], in0=t_ps[:], scalar1=din[:], scalar2=None, op0=Alu.mult
            )
            nc.vector.tensor_tensor(out=x[:], in0=x[:], in1=b_bc[:], op=Alu.add)
            nc.scalar.activation(out=x[:], in_=x[:], func=Act.Relu)
            f = ep.tile([P, D], f32, tag="f")
            nc.sync.dma_start(out=f[:bs, :], in_=feats_mine[j * P : j * P + bs, :])
            nc.vector.tensor_tensor(out=x[:bs, :], in0=x[:bs, :], in1=f[:bs, :], op=Alu.add)
            stats = ep.tile([P, 6], f32, tag="st")
            nc.vector.bn_stats(out=stats[:bs, :], in_=x[:bs, :])
            mv = ep.tile([P, 2], f32, tag="mv")
            nc.vector.bn_aggr(out=mv[:bs, :], in_=stats[:bs, :])
            sd = ep.tile([P, 1], f32, tag="sd")
            nc.scalar.activation(
                out=sd[:bs, :], in_=mv[:bs, 1:2], func=Act.Sqrt, bias=eps_sb[:bs, :]
            )
            nc.vector.reciprocal(out=sd[:bs, :], in_=sd[:bs, :])
            y = ep.tile([P, D], f32, tag="y")
            nc.vector.tensor_scalar(
                out=y[:bs, :],
                in0=x[:bs, :],
                scalar1=mv[:bs, 0:1],
                scalar2=sd[:bs, :],
                op0=Alu.subtract,
                op1=Alu.mult,
            )
            nc.vector.tensor_tensor(out=y[:bs, :], in0=y[:bs, :], in1=gamma_bc[:bs, :], op=Alu.mult)
            nc.vector.tensor_tensor(out=y[:bs, :], in0=y[:bs, :], in1=beta_bc[:bs, :], op=Alu.add)
            nc.sync.dma_start(out=out_t[j * P : j * P + bs, :], in_=y[:bs, :])

    if split_waits:
        _split_multiwaits(nc, mybir)
    return nc


# ---------------------------------------------------------------- entry point
def kernel(feats, src, dst, W, b, gamma, beta):
    _ensure_path()
    from concourse.bass_utils import run_bass_kernel_spmd

    n_cores = 8
    feats = np.asarray(feats, np.float32)
    in_maps, meta = host_prep(feats, src, dst, W, b, gamma, beta, n_cores)
    nc = build_nc(meta)
    res = run_bass_kernel_spmd(nc, in_maps, core_ids=list(range(n_cores)))
    out = np.concatenate([r["out"] for r in res.results], axis=0)
    return out[: meta["N"]].astype(np.float32)



# revision 13
# speedup vs baseline: 1.2826x; 1.2826x over previous
"""GCN layer (DGL GraphConv norm='both' + relu + residual + LayerNorm) on 8 trn2 NeuronCores.

v3: per-dst-block batched gathers via gpsimd.dma_gather (Q7 'mlp' library).
The fp16 h-table is viewed as pair rows [npad/2, 128] so indices fit int16
(idx = src//2 < 25024); each 128-edge tile is parity-pure (all-even or
all-odd src) so the matmul lhsT slices column 0 or 64 of the gathered pair.
One-hot scatter masks are generated in bulk with 0-stride broadcast APs.
Degree norm vectors are precomputed host-side.
"""

import numpy as np


def _ensure_path():
    try:
        import concourse  # noqa: F401
    except ImportError:
        import sys

        for p in ("/opt/trn_rl_repo", "/root/.axon_site/_ro/trn_rl_repo"):
            if p not in sys.path:
                sys.path.insert(0, p)


P = 128
LN_EPS = 1e-5


# ---------------------------------------------------------------- host prep
def host_prep(feats, src, dst, W, b, gamma, beta, n_cores):
    N, D = feats.shape
    assert N % n_cores == 0
    npc = N // n_cores                      # nodes per core
    nblk = (npc + P - 1) // P               # 128-node blocks per core
    rows_pp = (N + 2 + P - 1) // P          # table rows per partition
    npad = rows_pp * P                      # padded table rows (even, >= N+2)
    npair = npad // 2
    zero_pair = N // 2 if N % 2 == 0 else (N + 1) // 2
    # nodes N, N+1 are zero rows; pair N//2 is all-zero when N is even
    assert N % 2 == 0
    zero_pair = N // 2

    src = np.asarray(src).astype(np.int64)
    dst = np.asarray(dst).astype(np.int64)

    feats_pad = np.zeros((npad, D), np.float32)
    feats_pad[:N] = feats

    order = np.argsort(dst, kind="stable")
    src_s = src[order]
    dst_s = dst[order]
    rp_dst = np.searchsorted(dst_s, np.arange(N + 1)).astype(np.int64)

    # host-side degree vectors
    deg_out = np.bincount(src, minlength=npad)[:npad].astype(np.float64)
    deg_in = np.bincount(dst, minlength=N).astype(np.float64)
    dgo_val = (1.0 / np.sqrt(np.clip(deg_out, 1.0, None))).astype(np.float32)
    din_val = (1.0 / np.sqrt(np.clip(deg_in, 1.0, None))).astype(np.float32)
    # table layout is p-major: node p*rows_pp + r  ->  dgo[p, r]
    dgo = np.ascontiguousarray(dgo_val.reshape(P, rows_pp))

    # ---- per (core, block) parity-split edge lists -------------------
    # first pass: per-parity counts -> shared tile schedule
    ne = np.zeros((n_cores, nblk), np.int64)
    no = np.zeros((n_cores, nblk), np.int64)
    blk_edges = [[None] * nblk for _ in range(n_cores)]
    for m in range(n_cores):
        base = m * npc
        for j in range(nblk):
            lo = rp_dst[base + min(j * P, npc)]
            hi = rp_dst[base + min((j + 1) * P, npc)]
            s = src_s[lo:hi]
            d = dst_s[lo:hi] - (base + j * P)
            even = s % 2 == 0
            blk_edges[m][j] = (s[even], d[even], s[~even], d[~even])
            ne[m, j] = int(even.sum())
            no[m, j] = hi - lo - int(even.sum())
    Ke = (-(-ne // P)).max(axis=0).astype(np.int64)
    Ko = (-(-no // P)).max(axis=0).astype(np.int64)
    bump = (Ke + Ko) == 0
    Ke[bump] = 1
    K = Ke + Ko
    C = np.concatenate([[0], np.cumsum(K)]).astype(np.int64)
    tot_k = int(C[-1])
    kmax = int(K.max())

    # per-tile lhsT column offset: 0 for even-parity tiles, 64 for odd
    PAR = np.zeros(tot_k, np.int64)
    for j in range(nblk):
        PAR[C[j] + Ke[j] : C[j + 1]] = D

    idx16 = np.zeros((n_cores, P, tot_k * 8), np.int16)
    dstcol = np.full((n_cores, P, tot_k), -1.0, np.float16)
    for m in range(n_cores):
        for j in range(nblk):
            se, de, so, do_ = blk_edges[m][j]
            fi = np.full(int(K[j]) * P, zero_pair, np.int64)
            fd = np.full(int(K[j]) * P, -1.0, np.float32)
            fi[: len(se)] = se // 2
            fd[: len(se)] = de
            o0 = int(Ke[j]) * P
            fi[o0 : o0 + len(so)] = so // 2
            fd[o0 : o0 + len(so)] = do_
            # wrapped in 16 partitions AND replicated across the 8 Q7 cores
            # (each core reads its own 16-partition group)
            blk16 = fi.astype(np.int16).reshape(-1, 16).T
            idx16[m, :, C[j] * 8 : C[j + 1] * 8] = np.tile(blk16, (8, 1))
            dstcol[m, :, C[j] : C[j + 1]] = fd.reshape(int(K[j]), P).T.astype(np.float16)

    # per-core rsqrt(deg_in) in block layout: din_blk[p, j] = node j*P + p
    din_blk = np.ones((n_cores, P, nblk), np.float32)
    for m in range(n_cores):
        base = m * npc
        v = np.ones(nblk * P, np.float32)
        v[:npc] = din_val[base : base + npc]
        din_blk[m] = v.reshape(nblk, P).T

    iota = np.tile(np.arange(P, dtype=np.float16), (P, 1))
    ident = np.eye(D, dtype=np.float32)

    in_maps = []
    for m in range(n_cores):
        base = m * npc
        in_maps.append(
            {
                "feats_pad": feats_pad,
                "feats_mine": np.ascontiguousarray(feats[base : base + npc]).astype(np.float32),
                "idx16": np.ascontiguousarray(idx16[m]),
                "dstcol": np.ascontiguousarray(dstcol[m]),
                "dgo": dgo,
                "din_blk": np.ascontiguousarray(din_blk[m]),
                "Wmat": np.asarray(W, np.float32),
                "bvec": np.asarray(b, np.float32),
                "gamma": np.asarray(gamma, np.float32),
                "beta": np.asarray(beta, np.float32),
                "iota": iota,
                "ident": ident,
            }
        )

    meta = dict(
        N=N, D=D, n_cores=n_cores, npc=npc, nblk=nblk, rows_pp=rows_pp,
        npad=npad, npair=npair, K=[int(k) for k in K], C=[int(c) for c in C],
        tot_k=tot_k, kmax=kmax, PAR=[int(p) for p in PAR],
    )
    return in_maps, meta


def _split_multiwaits(nc, mybir):
    """This walrus build allows only one sync-wait per instruction; hoist
    extra waits onto same-engine NoOps placed just before the instruction."""
    n = 0
    for f in nc.m.functions:
        for bb in f.blocks:
            newlist = []
            for inst in bb.instructions:
                si = getattr(inst, "sync_info", None)
                if si is not None and len(si.on_wait) > 1:
                    waits = list(si.on_wait)
                    for w in waits[:-1]:
                        nop = mybir.InstNoOp(name=f"I-WS-{n}", ins=[], outs=[])
                        n += 1
                        nop.engine = inst.engine
                        nop.sync_info = mybir.SyncInfo(on_wait=[w], on_update=[])
                        newlist.append(nop)
                    inst.sync_info = mybir.SyncInfo(
                        on_wait=[waits[-1]], on_update=list(si.on_update)
                    )
                newlist.append(inst)
            bb.instructions = newlist


# ---------------------------------------------------------------- device program
def build_nc(meta, debug=False, split_waits=True):
    _ensure_path()
    from contextlib import ExitStack

    import concourse.bass as bass
    import concourse.tile as tile
    from concourse import library_config, mybir

    dt = mybir.dt
    f32, f16, i16 = dt.float32, dt.float16, dt.int16
    Alu = mybir.AluOpType
    Act = mybir.ActivationFunctionType

    N = meta["N"]
    D = meta["D"]
    npc = meta["npc"]
    nblk = meta["nblk"]
    rows_pp = meta["rows_pp"]
    npad = meta["npad"]
    npair = meta["npair"]
    K = meta["K"]
    C = meta["C"]
    tot_k = meta["tot_k"]
    kmax = meta["kmax"]
    PAR = meta["PAR"]

    nc = bass.Bass()

    feats_pad = nc.declare_dram_parameter("feats_pad", [npad, D], f32, isOutput=False)
    feats_mine = nc.declare_dram_parameter("feats_mine", [npc, D], f32, isOutput=False)
    idx16_in = nc.declare_dram_parameter("idx16", [P, tot_k * 8], i16, isOutput=False)
    dstcol_in = nc.declare_dram_parameter("dstcol", [P, tot_k], f16, isOutput=False)
    dgo_in = nc.declare_dram_parameter("dgo", [P, rows_pp], f32, isOutput=False)
    din_in = nc.declare_dram_parameter("din_blk", [P, nblk], f32, isOutput=False)
    W_in = nc.declare_dram_parameter("Wmat", [D, D], f32, isOutput=False)
    b_in = nc.declare_dram_parameter("bvec", [D], f32, isOutput=False)
    gamma_in = nc.declare_dram_parameter("gamma", [D], f32, isOutput=False)
    beta_in = nc.declare_dram_parameter("beta", [D], f32, isOutput=False)
    iota_in = nc.declare_dram_parameter("iota", [P, P], f16, isOutput=False)
    ident_in = nc.declare_dram_parameter("ident", [D, D], f32, isOutput=False)
    out_t = nc.declare_dram_parameter("out", [npc, D], f32, isOutput=True)

    h_dram = nc.dram_tensor("h_table", [npad, D], f16)

    def bcast_row(ap, parts):
        # [L] dram vector -> [parts, L] partition-broadcast access pattern
        return bass.AP(tensor=ap.tensor, offset=ap.offset, ap=[[0, parts]] + list(ap.ap))

    def bcast_mid(ap, reps):
        # [P, L] -> [P, reps, L] via 0-stride middle dim
        return bass.AP(tensor=ap.tensor, offset=ap.offset,
                       ap=[ap.ap[0], [0, reps], ap.ap[1]])

    def bcast_inner(ap, reps):
        # [P, L] -> [P, L, reps] via 0-stride inner dim
        return bass.AP(tensor=ap.tensor, offset=ap.offset,
                       ap=[ap.ap[0], ap.ap[1], [0, reps]])

    # h table viewed as pair rows: [npair, 2*D] fp16 (256B rows)
    pair_view = bass.AP(tensor=h_dram[:].tensor, offset=0, ap=[[2 * D, npair], [1, 2 * D]])

    with ExitStack() as ctx:
        tc = ctx.enter_context(tile.TileContext(nc))
        const = ctx.enter_context(tc.tile_pool(name="const", bufs=1))
        ph1 = ctx.enter_context(tc.tile_pool(name="ph1", bufs=2))
        gp = ctx.enter_context(tc.tile_pool(name="gp", bufs=3))
        indp = ctx.enter_context(tc.tile_pool(name="indp", bufs=3))
        ep = ctx.enter_context(tc.tile_pool(name="ep", bufs=3))
        pp = ctx.enter_context(tc.tile_pool(name="pp", bufs=2, space="PSUM"))

        nc.gpsimd.load_library(library_config.mlp)

        # one shared register per distinct tile count (register pool is small)
        nidx_regs = {}
        for kj in sorted(set(K)):
            nidx_regs[kj] = nc.gpsimd.to_reg(kj * P)

        # ---- constants -------------------------------------------------
        iota_sb = const.tile([P, P], f16)
        nc.sync.dma_start(out=iota_sb[:], in_=iota_in[:])
        ident_sb = const.tile([D, D], f32)
        nc.sync.dma_start(out=ident_sb[:], in_=ident_in[:])
        w_f32 = const.tile([D, D], f32)
        nc.sync.dma_start(out=w_f32[:], in_=W_in[:])
        w_sb = const.tile([D, D], f16)
        nc.vector.tensor_copy(out=w_sb[:], in_=w_f32[:])
        b_bc = const.tile([P, D], f32)
        nc.sync.dma_start(out=b_bc[:], in_=bcast_row(b_in[:], P))
        gamma_bc = const.tile([P, D], f32)
        nc.sync.dma_start(out=gamma_bc[:], in_=bcast_row(gamma_in[:], P))
        beta_bc = const.tile([P, D], f32)
        nc.sync.dma_start(out=beta_bc[:], in_=bcast_row(beta_in[:], P))
        eps_sb = const.tile([P, 1], f32)
        nc.vector.memset(eps_sb[:], LN_EPS)
        idx16_sb = const.tile([P, tot_k * 8], i16)
        nc.sync.dma_start(out=idx16_sb[:], in_=idx16_in[:])
        dstcol_sb = const.tile([P, tot_k], f16)
        nc.sync.dma_start(out=dstcol_sb[:], in_=dstcol_in[:])
        dgo_sb = const.tile([P, rows_pp], f32)
        nc.sync.dma_start(out=dgo_sb[:], in_=dgo_in[:])
        din_sb = const.tile([P, nblk], f32)
        nc.sync.dma_start(out=din_sb[:], in_=din_in[:])

        # ---- phase 1: h table = fp16(feats * rsqrt(deg_out)) -----------
        fview = feats_pad[:].rearrange("(p r) d -> p r d", p=P)
        hview = h_dram[:].rearrange("(p r) d -> p r d", p=P)
        n_chunks = 8
        cw = -(-rows_pp // n_chunks)
        for c in range(n_chunks):
            r0 = c * cw
            r1 = min(r0 + cw, rows_pp)
            if r0 >= r1:
                break
            w_ = r1 - r0
            ft = ph1.tile([P, cw, D], f32, tag="ft")
            nc.sync.dma_start(out=ft[:, :w_, :], in_=fview[:, r0:r1, :])
            ht = ph1.tile([P, cw, D], f16, tag="ht")
            nc.vector.tensor_tensor(
                out=ht[:, :w_, :],
                in0=ft[:, :w_, :],
                in1=bcast_inner(dgo_sb[:, r0:r1], D),
                op=Alu.mult,
            )
            nc.sync.dma_start(out=hview[:, r0:r1, :], in_=ht[:, :w_, :])

        tc.strict_bb_all_engine_barrier()

        # ---- phase 2: per-block batched gather + aggregation -----------
        for j in range(nblk):
            kj = K[j]
            bs = min(P, npc - j * P)

            g_blk = gp.tile([P, kmax, 2 * D], f16, tag="g")
            nc.gpsimd.dma_gather(
                out_ap=g_blk[:, :kj, :],
                in_ap=pair_view,
                idxs_ap=idx16_sb[:, C[j] * 8 : C[j + 1] * 8],
                num_idxs=kj * P,
                num_idxs_reg=nidx_regs[kj],
                elem_size=2 * D,
                single_packet=False,
            )

            ind_blk = indp.tile([P, kmax, P], f16, tag="ind")
            nc.vector.tensor_tensor(
                out=ind_blk[:, :kj, :],
                in0=bcast_mid(iota_sb[:], kj),
                in1=bcast_inner(dstcol_sb[:, C[j] : C[j + 1]], P),
                op=Alu.is_equal,
            )

            agg_ps = pp.tile([D, P], f32, tag="agg")
            for k in range(kj):
                off = PAR[C[j] + k]
                nc.tensor.matmul(
                    out=agg_ps[:],
                    lhsT=g_blk[:, k, off : off + D],
                    rhs=ind_blk[:, k, :],
                    start=(k == 0),
                    stop=(k == kj - 1),
                )

            agg_sb = ep.tile([D, P], f16, tag="aggsb")
            nc.vector.tensor_copy(out=agg_sb[:], in_=agg_ps[:])
            w_ps = pp.tile([D, P], f32, tag="wps")
            nc.tensor.matmul(out=w_ps[:], lhsT=w_sb[:], rhs=agg_sb[:], start=True, stop=True)
            w_sbuf = ep.tile([D, P], f32, tag="wsb")
            nc.vector.tensor_copy(out=w_sbuf[:], in_=w_ps[:])
            t_ps = pp.tile([P, D], f32, tag="tps")
            nc.tensor.transpose(out=t_ps[:], in_=w_sbuf[:], identity=ident_sb[:])

            x = ep.tile([P, D], f32, tag="x")
            nc.vector.tensor_scalar(
                out=x[:], in0=t_ps[:], scalar1=din_sb[:, j : j + 1], scalar2=None,
                op0=Alu.mult,
            )
            nc.vector.tensor_tensor(out=x[:], in0=x[:], in1=b_bc[:], op=Alu.add)
            nc.scalar.activation(out=x[:], in_=x[:], func=Act.Relu)
            f = ep.tile([P, D], f32, tag="f")
            nc.sync.dma_start(out=f[:bs, :], in_=feats_mine[j * P : j * P + bs, :])
            nc.vector.tensor_tensor(out=x[:bs, :], in0=x[:bs, :], in1=f[:bs, :], op=Alu.add)
            stats = ep.tile([P, 6], f32, tag="st")
            nc.vector.bn_stats(out=stats[:bs, :], in_=x[:bs, :])
            mv = ep.tile([P, 2], f32, tag="mv")
            nc.vector.bn_aggr(out=mv[:bs, :], in_=stats[:bs, :])
            sd = ep.tile([P, 1], f32, tag="sd")
            nc.scalar.activation(
                out=sd[:bs, :], in_=mv[:bs, 1:2], func=Act.Sqrt, bias=eps_sb[:bs, :]
            )
            nc.vector.reciprocal(out=sd[:bs, :], in_=sd[:bs, :])
            y = ep.tile([P, D], f32, tag="y")
            nc.vector.tensor_scalar(
                out=y[:bs, :],
                in0=x[:bs, :],
                scalar1=mv[:bs, 0:1],
                scalar2=sd[:bs, :],
                op0=Alu.subtract,
                op1=Alu.mult,
            )
            nc.vector.tensor_tensor(out=y[:bs, :], in0=y[:bs, :], in1=gamma_bc[:bs, :], op=Alu.mult)
            nc.vector.tensor_tensor(out=y[:bs, :], in0=y[:bs, :], in1=beta_bc[:bs, :], op=Alu.add)
            nc.sync.dma_start(out=out_t[j * P : j * P + bs, :], in_=y[:bs, :])

    # walrus requires the 16-word PSEUDO_INST encoding on the library-reload
    # pseudo instruction; bass leaves instr empty, so pack it here (post
    # tile-scheduling, on the final instruction objects).
    import concourse.bass_isa as bass_isa

    for f in nc.m.functions:
        for bb in f.blocks:
            for inst in bb.instructions:
                if isinstance(inst, bass_isa.InstPseudoReloadLibraryIndex):
                    words, _ = bass_isa.isa_struct(
                        nc.isa,
                        nc.isa.Opcode.NEURON_ISA_TPB_OPCODE_PSEUDO_INST,
                        {"pseudo_opcode": 2, "lib_index": inst.lib_index},
                        struct_name="NEURON_ISA_TPB_PSEUDO_LIBRARY_RELOAD_INDEX_STRUCT",
                    )
                    inst.instr = words

    if split_waits:
        _split_multiwaits(nc, mybir)
    return nc


# ---------------------------------------------------------------- entry point
def kernel(feats, src, dst, W, b, gamma, beta):
    _ensure_path()
    from concourse.bass_utils import run_bass_kernel_spmd

    n_cores = 8
    feats = np.asarray(feats, np.float32)
    in_maps, meta = host_prep(feats, src, dst, W, b, gamma, beta, n_cores)
    nc = build_nc(meta)
    res = run_bass_kernel_spmd(nc, in_maps, core_ids=list(range(n_cores)))
    out = np.concatenate([r["out"] for r in res.results], axis=0)
    return out[: meta["N"]].astype(np.float32)
